# revision 36
# baseline (speedup 1.0000x reference)
"""Distortion loss (mip-NeRF 360 style) on 8 Trainium2 NeuronCores.

Math: for each ray with sorted interval boundaries t (N+1 values given as
intervals (t_i, t_{i+1})), s = (t - t_near) / (t_far - t_near),
  u_i   = (s_i + s_{i+1}) / 2           (midpoints, SORTED because t sorted)
  loss  = sum_ij w_i w_j |u_i - u_j| + (1/3) sum_i w_i^2 (s_{i+1} - s_i)

Because u is sorted along N, the O(N^2) pairwise term collapses to O(N).
With m = 2u = s0 + s1 and S_ij = sign(i - j):
  inter = sum_ij w_i m_i sign(i-j) w_j = sum_i (w m)_i (S w)_i
(S w computed on the tensor engine via a constant sign matrix; the
equivalent prefix-sum form via tensor_tensor_scan is the "scan" variant).
s-space affine rescaling factors out entirely:
  loss = inv * (inter_t + intra_t / 3),  inv = 1 / (t_far - t_near)
so everything is computed in t-space with one final per-ray scale.

Sharding: embarrassingly data-parallel over rays; B=4096 rays split into 8
shards of 512; each core processes 4 groups of 128 rays (128 partitions),
pipelined: per-group DMA loads (t_inters on the sync HWDGE ring, weights on
the scalar ring) overlap per-group compute spread across GPSIMD (m, du),
ACT (w^2, PSUM copies), PE (transpose + sign-matmul in fp32r), DVE
(w*m, fused multiply-reduce via scalar_tensor_tensor accum).
"""

import numpy as np

B, N = 4096, 128
NCORES = 8
BS = B // NCORES  # 512 rays per core
P = 128  # partitions
G = BS // P  # 4 ray-groups per core

# "scan": DVE tensor_tensor_scan prefix sums.
# "matmul": PE sign-matrix matmul for the pairwise term (fp32r single-pass).
# "matmul_f32": same but full-precision fp32 matmuls.
VARIANT = "matmul"

_CACHE = {}


def _build(variant):
    from contextlib import ExitStack

    import concourse.bacc as bacc
    import concourse.mybir as mybir
    import concourse.tile as tile
    from concourse.masks import make_identity

    ALU = mybir.AluOpType
    FP32 = mybir.dt.float32
    FP32R = mybir.dt.float32r

    nc = bacc.Bacc("TRN2", target_bir_lowering=False, debug=False)

    t_d = nc.dram_tensor("t_inters", [BS, N, 2], FP32, kind="ExternalInput")
    w_d = nc.dram_tensor("weights", [BS, N], FP32, kind="ExternalInput")
    tn_d = nc.dram_tensor("t_near", [BS, 1], FP32, kind="ExternalInput")
    tf_d = nc.dram_tensor("t_far", [BS, 1], FP32, kind="ExternalInput")
    o_d = nc.dram_tensor("out", [BS], FP32, kind="ExternalOutput")

    t_v = t_d.ap().rearrange("(g p) n k -> g p (n k)", p=P)  # [G, P, 256]
    w_v = w_d.ap().rearrange("(g p) n -> g p n", p=P)  # [G, P, 128]

    with tile.TileContext(nc) as tc, ExitStack() as ctx:
        pool = ctx.enter_context(tc.tile_pool(name="main", bufs=1))
        scr_pool = ctx.enter_context(tc.tile_pool(name="scr", bufs=4))
        psum = ctx.enter_context(tc.tile_pool(name="psum", bufs=2, space="PSUM"))
        psum_sw = ctx.enter_context(tc.tile_pool(name="psum_sw", bufs=4, space="PSUM"))
        psum1 = ctx.enter_context(tc.tile_pool(name="psum1", bufs=1, space="PSUM"))

        # ---- per-group loads (ray index = g*128 + p) ----
        # t-groups on the sync HWDGE ring, w-groups on the scalar ring so the
        # two streams issue and transfer concurrently.
        t_tiles, w_tiles = [], []
        for g in range(G):
            wt = pool.tile([P, N], FP32, tag=f"w{g}")
            nc.scalar.dma_start(wt[:], w_v[g])
            w_tiles.append(wt)
            tt = pool.tile([P, N, 2], FP32, tag=f"t{g}")
            nc.sync.dma_start(tt[:].rearrange("p n k -> p (n k)"), t_v[g])
            t_tiles.append(tt)
        tnT = pool.tile([G, P], FP32)
        nc.sync.dma_start(tnT[:], tn_d.ap().rearrange("(g p) one -> g (p one)", g=G))
        tfT = pool.tile([G, P], FP32)
        nc.sync.dma_start(tfT[:], tf_d.ap().rearrange("(g p) one -> g (p one)", g=G))

        # ---- constants (GPSIMD, overlapped with loads) ----
        identity = pool.tile([P, P], FP32)
        make_identity(nc, identity[:])
        sg = pool.tile([P, P], FP32)
        nc.gpsimd.memset(sg[:], -1.0)
        # keep -1.0 where (p - f) >= 0 i.e. f <= p; else fill +1
        nc.gpsimd.affine_select(
            out=sg[:], in_=sg[:], compare_op=ALU.is_ge, fill=1.0,
            base=0, pattern=[[-1, P]], channel_multiplier=1,
        )
        # zero the diagonal: keep where (p - f) != 0
        nc.gpsimd.affine_select(
            out=sg[:], in_=sg[:], compare_op=ALU.not_equal, fill=0.0,
            base=0, pattern=[[-1, P]], channel_multiplier=1,
        )
        sg_r = pool.tile([P, P], FP32R)
        nc.vector.tensor_copy(sg_r[:], sg[:])  # DVE rounds to fp32r, runs early

        loss_cols = pool.tile([P, G], FP32)
        accA = pool.tile([P, G], FP32)
        accB = pool.tile([P, G], FP32)
        accC = pool.tile([P, G], FP32)

        use_f32r = variant == "matmul"
        mm_dt = FP32R if use_f32r else FP32

        if variant == "scan":
            zeros = pool.tile([P, N], FP32)
            nc.vector.memset(zeros[:], 0.0)

        du_tiles, w2_tiles = [], []
        for g in range(G):
            tt, wt = t_tiles[g], w_tiles[g]
            s0 = tt[:, :, 0]  # [P, N] interval starts (t-space)
            s1 = tt[:, :, 1]
            m_g = scr_pool.tile([P, N], FP32, tag="m")
            nc.gpsimd.tensor_tensor(m_g[:], s0, s1, ALU.add)  # m = s0+s1 = 2u
            du_g = scr_pool.tile([P, N], FP32, tag=f"du{g}")
            nc.gpsimd.tensor_tensor(du_g[:], s1, s0, ALU.subtract)
            du_tiles.append(du_g)
            w2_g = scr_pool.tile([P, N], FP32, tag=f"w2{g}")
            nc.scalar.square(w2_g[:], wt[:])
            w2_tiles.append(w2_g)
            wm_g = scr_pool.tile([P, N], FP32, tag="wm")
            nc.vector.tensor_tensor(wm_g[:], wt[:], m_g[:], ALU.mult)
            scr3 = scr_pool.tile([P, N], FP32, tag="scr")
            nc.vector.scalar_tensor_tensor(
                out=scr3[:], in0=w2_g[:], scalar=1.0 / 3.0, in1=du_g[:],
                op0=ALU.mult, op1=ALU.mult, accum_out=accC[:, g : g + 1],
            )

            if variant == "scan":
                cumW = scr_pool.tile([P, N], FP32, tag="cumW")
                nc.vector.tensor_tensor_scan(
                    cumW[:], wt[:], zeros[:], 0.0, ALU.add, ALU.add
                )
                cumWM = scr_pool.tile([P, N], FP32, tag="cumWM")
                nc.vector.tensor_tensor_scan(
                    cumWM[:], wm_g[:], zeros[:], 0.0, ALU.add, ALU.add
                )
                scr = scr_pool.tile([P, N], FP32, tag="scr")
                nc.vector.scalar_tensor_tensor(
                    out=scr[:], in0=wm_g[:], scalar=1.0, in1=cumW[:],
                    op0=ALU.mult, op1=ALU.mult, accum_out=accA[:, g : g + 1],
                )
                scr2 = scr_pool.tile([P, N], FP32, tag="scr")
                nc.vector.scalar_tensor_tensor(
                    out=scr2[:], in0=wt[:], scalar=1.0, in1=cumWM[:],
                    op0=ALU.mult, op1=ALU.mult, accum_out=accB[:, g : g + 1],
                )
            else:
                wT_ps = psum.tile([P, P], FP32, tag="wT")
                nc.tensor.transpose(wT_ps[:], wt[:], identity[:])
                wT = scr_pool.tile([P, P], mm_dt, tag="wT_sb")
                nc.scalar.copy(wT[:], wT_ps[:])  # ACT rounds to fp32r
                sw_ps = psum_sw.tile([P, P], FP32, tag="sw")
                nc.tensor.matmul(
                    sw_ps[:], wT[:], sg_r[:] if use_f32r else sg[:]
                )  # sw[b, i] = sum_j sign(i-j) w[b, j]
                scr = scr_pool.tile([P, N], FP32, tag="scr")
                nc.vector.scalar_tensor_tensor(
                    out=scr[:], in0=wm_g[:], scalar=1.0, in1=sw_ps[:],
                    op0=ALU.mult, op1=ALU.mult, accum_out=accA[:, g : g + 1],
                )

        # ---- combine partial sums -> per-ray loss, scale, transpose, store
        # inv = 1/(t_far - t_near) in [P, G] layout: [4,128] reciprocal runs
        # on 4 DVE lanes (~1us); transposing the denominator first makes it
        # a [128,4] op. Emitted after the loop so the tf-gated transpose
        # can't stall the in-order PE stream. ----
        dT = scr_pool.tile([G, P], FP32)
        nc.vector.tensor_tensor(dT[:], tfT[:], tnT[:], ALU.subtract)
        d128_ps = psum1.tile([P, G], FP32, tag="d128")
        nc.tensor.transpose(d128_ps[:], dT[:], identity[:G, :G])
        inv128 = pool.tile([P, G], FP32)
        nc.vector.reciprocal(inv128[:], d128_ps[:])

        if variant == "scan":
            # loss = A - B + C
            nc.vector.scalar_tensor_tensor(
                out=loss_cols[:], in0=accB[:], scalar=-1.0, in1=accA[:],
                op0=ALU.mult, op1=ALU.add,
            )
            nc.vector.tensor_tensor(loss_cols[:], loss_cols[:], accC[:], ALU.add)
            nc.vector.tensor_tensor(loss_cols[:], loss_cols[:], inv128[:], ALU.mult)
        else:
            # loss = (A + C) * inv
            nc.vector.tensor_tensor(loss_cols[:], accA[:], accC[:], ALU.add)
            nc.vector.tensor_tensor(loss_cols[:], loss_cols[:], inv128[:], ALU.mult)

        lossT_ps = psum1.tile([G, P], FP32, tag="lossT")
        nc.tensor.transpose(lossT_ps[:], loss_cols[:], identity[:])
        outT = pool.tile([G, P], FP32)
        nc.vector.tensor_copy(outT[:], lossT_ps[:])
        nc.sync.dma_start(
            o_d.ap().rearrange("(g p) -> g p", g=G), outT[:], single_packet=True
        )

    nc.compile()
    return nc


def _get_nc(variant=None):
    variant = variant or VARIANT
    if variant not in _CACHE:
        _CACHE[variant] = _build(variant)
    return _CACHE[variant]


def kernel(t_inters, weights, t_near, t_far, _variant=None, _trace=False):
    from concourse.bass_utils import run_bass_kernel_spmd

    nc = _get_nc(_variant)

    t_inters = np.ascontiguousarray(np.asarray(t_inters, dtype=np.float32))
    weights = np.ascontiguousarray(np.asarray(weights, dtype=np.float32))
    t_near = np.ascontiguousarray(np.asarray(t_near, dtype=np.float32))
    t_far = np.ascontiguousarray(np.asarray(t_far, dtype=np.float32))

    in_maps = []
    for c in range(NCORES):
        s = slice(c * BS, (c + 1) * BS)
        in_maps.append(
            {
                "t_inters": t_inters[s],
                "weights": weights[s],
                "t_near": t_near[s],
                "t_far": t_far[s],
            }
        )

    res = run_bass_kernel_spmd(nc, in_maps, core_ids=list(range(NCORES)), trace=_trace)
    out = np.concatenate([res.results[c]["out"] for c in range(NCORES)])
    if _trace:
        return out, res
    return out


# revision 38
# speedup vs baseline: 1.0405x; 1.0405x over previous
"""Distortion loss (mip-NeRF 360 style) on 8 Trainium2 NeuronCores.

Math: for each ray with sorted interval boundaries t (N+1 values given as
intervals (t_i, t_{i+1})), s = (t - t_near) / (t_far - t_near),
  u_i   = (s_i + s_{i+1}) / 2           (midpoints, SORTED because t sorted)
  loss  = sum_ij w_i w_j |u_i - u_j| + (1/3) sum_i w_i^2 (s_{i+1} - s_i)

Because u is sorted along N, the O(N^2) pairwise term collapses to O(N).
With m = 2u = s0 + s1 and S_ij = sign(i - j):
  inter = sum_ij w_i m_i sign(i-j) w_j = sum_i (w m)_i (S w)_i
(S w computed on the tensor engine via a constant sign matrix; the
equivalent prefix-sum form via tensor_tensor_scan is the "scan" variant).
s-space affine rescaling factors out entirely:
  loss = inv * (inter_t + intra_t / 3),  inv = 1 / (t_far - t_near)
so everything is computed in t-space with one final per-ray scale.

Sharding: embarrassingly data-parallel over rays; B=4096 rays split into 8
shards of 512; each core processes 4 groups of 128 rays (128 partitions),
pipelined: per-group DMA loads (t_inters on the sync HWDGE ring, weights on
the scalar ring) overlap per-group compute spread across GPSIMD (m, du),
ACT (w^2, PSUM copies), PE (transpose + sign-matmul in fp32r), DVE
(w*m, fused multiply-reduce via scalar_tensor_tensor accum).
"""

import numpy as np

B, N = 4096, 128
NCORES = 8
BS = B // NCORES  # 512 rays per core
P = 128  # partitions
G = BS // P  # 4 ray-groups per core

# "scan": DVE tensor_tensor_scan prefix sums.
# "matmul": PE sign-matrix matmul for the pairwise term (fp32r single-pass).
# "matmul_f32": same but full-precision fp32 matmuls.
VARIANT = "matmul"

_CACHE = {}


def _build(variant):
    from contextlib import ExitStack

    import concourse.bacc as bacc
    import concourse.mybir as mybir
    import concourse.tile as tile
    from concourse.masks import make_identity

    ALU = mybir.AluOpType
    FP32 = mybir.dt.float32
    FP32R = mybir.dt.float32r

    nc = bacc.Bacc("TRN2", target_bir_lowering=False, debug=False)

    t_d = nc.dram_tensor("t_inters", [BS, N, 2], FP32, kind="ExternalInput")
    w_d = nc.dram_tensor("weights", [BS, N], FP32, kind="ExternalInput")
    tn_d = nc.dram_tensor("t_near", [BS, 1], FP32, kind="ExternalInput")
    tf_d = nc.dram_tensor("t_far", [BS, 1], FP32, kind="ExternalInput")
    o_d = nc.dram_tensor("out", [BS], FP32, kind="ExternalOutput")

    t_v = t_d.ap().rearrange("(g p) n k -> g p (n k)", p=P)  # [G, P, 256]
    w_v = w_d.ap().rearrange("(g p) n -> g p n", p=P)  # [G, P, 128]

    with tile.TileContext(nc) as tc, ExitStack() as ctx:
        pool = ctx.enter_context(tc.tile_pool(name="main", bufs=1))
        scr_pool = ctx.enter_context(tc.tile_pool(name="scr", bufs=4))
        psum = ctx.enter_context(tc.tile_pool(name="psum", bufs=2, space="PSUM"))
        psum_sw = ctx.enter_context(tc.tile_pool(name="psum_sw", bufs=4, space="PSUM"))
        psum1 = ctx.enter_context(tc.tile_pool(name="psum1", bufs=1, space="PSUM"))

        # ---- per-group loads (ray index = g*128 + p) ----
        # t-groups on the sync HWDGE ring, w-groups on the scalar ring so the
        # two streams issue and transfer concurrently.
        t_tiles, w_tiles = [], []
        for g in range(G):
            wt = pool.tile([P, N], FP32, tag=f"w{g}")
            nc.scalar.dma_start(wt[:], w_v[g])
            w_tiles.append(wt)
            tt = pool.tile([P, N, 2], FP32, tag=f"t{g}")
            nc.sync.dma_start(tt[:].rearrange("p n k -> p (n k)"), t_v[g])
            t_tiles.append(tt)
        tnT = pool.tile([G, P], FP32)
        nc.sync.dma_start(tnT[:], tn_d.ap().rearrange("(g p) one -> g (p one)", g=G))
        tfT = pool.tile([G, P], FP32)
        nc.sync.dma_start(tfT[:], tf_d.ap().rearrange("(g p) one -> g (p one)", g=G))

        # ---- constants (GPSIMD, overlapped with loads) ----
        identity = pool.tile([P, P], FP32)
        make_identity(nc, identity[:])
        sg = pool.tile([P, P], FP32)
        nc.gpsimd.memset(sg[:], -1.0)
        # keep -1.0 where (p - f) >= 0 i.e. f <= p; else fill +1
        nc.gpsimd.affine_select(
            out=sg[:], in_=sg[:], compare_op=ALU.is_ge, fill=1.0,
            base=0, pattern=[[-1, P]], channel_multiplier=1,
        )
        # zero the diagonal: keep where (p - f) != 0
        nc.gpsimd.affine_select(
            out=sg[:], in_=sg[:], compare_op=ALU.not_equal, fill=0.0,
            base=0, pattern=[[-1, P]], channel_multiplier=1,
        )
        sg_r = pool.tile([P, P], FP32R)
        nc.vector.tensor_copy(sg_r[:], sg[:])  # DVE rounds to fp32r, runs early

        loss_cols = pool.tile([P, G], FP32)
        accA = pool.tile([P, G], FP32)
        accB = pool.tile([P, G], FP32)
        accC = pool.tile([P, G], FP32)

        use_f32r = variant == "matmul"
        mm_dt = FP32R if use_f32r else FP32

        if variant == "scan":
            zeros = pool.tile([P, N], FP32)
            nc.vector.memset(zeros[:], 0.0)

        du_tiles, w2_tiles = [], []
        for g in range(G):
            tt, wt = t_tiles[g], w_tiles[g]
            s0 = tt[:, :, 0]  # [P, N] interval starts (t-space)
            s1 = tt[:, :, 1]
            m_g = scr_pool.tile([P, N], FP32, tag="m")
            nc.gpsimd.tensor_tensor(m_g[:], s0, s1, ALU.add)  # m = s0+s1 = 2u
            du_g = scr_pool.tile([P, N], FP32, tag=f"du{g}")
            nc.gpsimd.tensor_tensor(du_g[:], s1, s0, ALU.subtract)
            du_tiles.append(du_g)
            w2_g = scr_pool.tile([P, N], FP32, tag=f"w2{g}")
            nc.scalar.square(w2_g[:], wt[:])
            w2_tiles.append(w2_g)
            wm_g = scr_pool.tile([P, N], FP32, tag="wm")
            nc.vector.tensor_tensor(wm_g[:], wt[:], m_g[:], ALU.mult)
            scr3 = scr_pool.tile([P, N], FP32, tag="scr")
            nc.vector.scalar_tensor_tensor(
                out=scr3[:], in0=w2_g[:], scalar=1.0 / 3.0, in1=du_g[:],
                op0=ALU.mult, op1=ALU.mult, accum_out=accC[:, g : g + 1],
            )

            if variant == "scan":
                cumW = scr_pool.tile([P, N], FP32, tag="cumW")
                nc.vector.tensor_tensor_scan(
                    cumW[:], wt[:], zeros[:], 0.0, ALU.add, ALU.add
                )
                cumWM = scr_pool.tile([P, N], FP32, tag="cumWM")
                nc.vector.tensor_tensor_scan(
                    cumWM[:], wm_g[:], zeros[:], 0.0, ALU.add, ALU.add
                )
                scr = scr_pool.tile([P, N], FP32, tag="scr")
                nc.vector.scalar_tensor_tensor(
                    out=scr[:], in0=wm_g[:], scalar=1.0, in1=cumW[:],
                    op0=ALU.mult, op1=ALU.mult, accum_out=accA[:, g : g + 1],
                )
                scr2 = scr_pool.tile([P, N], FP32, tag="scr")
                nc.vector.scalar_tensor_tensor(
                    out=scr2[:], in0=wt[:], scalar=1.0, in1=cumWM[:],
                    op0=ALU.mult, op1=ALU.mult, accum_out=accB[:, g : g + 1],
                )
            else:
                wT_ps = psum.tile([P, P], FP32, tag="wT")
                nc.tensor.transpose(wT_ps[:], wt[:], identity[:])
                wT = scr_pool.tile([P, P], mm_dt, tag="wT_sb")
                nc.scalar.copy(wT[:], wT_ps[:])  # ACT rounds to fp32r
                sw_ps = psum_sw.tile([P, P], FP32, tag="sw")
                nc.tensor.matmul(
                    sw_ps[:], wT[:], sg_r[:] if use_f32r else sg[:]
                )  # sw[b, i] = sum_j sign(i-j) w[b, j]
                scr = scr_pool.tile([P, N], FP32, tag="scr")
                nc.vector.scalar_tensor_tensor(
                    out=scr[:], in0=wm_g[:], scalar=1.0, in1=sw_ps[:],
                    op0=ALU.mult, op1=ALU.mult, accum_out=accA[:, g : g + 1],
                )

        # ---- combine partial sums -> per-ray loss, scale, transpose, store
        # inv = 1/(t_far - t_near) in [P, G] layout: [4,128] reciprocal runs
        # on 4 DVE lanes (~1us); transposing the denominator first makes it
        # a [128,4] op. Emitted after the loop so the tf-gated transpose
        # can't stall the in-order PE stream. ----
        dT = scr_pool.tile([G, P], FP32)
        nc.gpsimd.tensor_tensor(dT[:], tfT[:], tnT[:], ALU.subtract)
        d128_ps = psum1.tile([P, G], FP32, tag="d128")
        nc.tensor.transpose(d128_ps[:], dT[:], identity[:G, :G])
        inv128 = pool.tile([P, G], FP32)
        nc.vector.reciprocal(inv128[:], d128_ps[:])

        if variant == "scan":
            # loss = A - B + C
            nc.vector.scalar_tensor_tensor(
                out=loss_cols[:], in0=accB[:], scalar=-1.0, in1=accA[:],
                op0=ALU.mult, op1=ALU.add,
            )
            nc.vector.tensor_tensor(loss_cols[:], loss_cols[:], accC[:], ALU.add)
            nc.vector.tensor_tensor(loss_cols[:], loss_cols[:], inv128[:], ALU.mult)
        else:
            # loss = (A + C) * inv
            nc.vector.tensor_tensor(loss_cols[:], accA[:], accC[:], ALU.add)
            nc.vector.tensor_tensor(loss_cols[:], loss_cols[:], inv128[:], ALU.mult)

        lossT_ps = psum1.tile([G, P], FP32, tag="lossT")
        nc.tensor.transpose(lossT_ps[:], loss_cols[:], identity[:])
        outT = pool.tile([G, P], FP32)
        nc.vector.tensor_copy(outT[:], lossT_ps[:])
        nc.sync.dma_start(o_d.ap().rearrange("(g p) -> g p", g=G), outT[:])

    nc.compile()
    return nc


def _get_nc(variant=None):
    variant = variant or VARIANT
    if variant not in _CACHE:
        _CACHE[variant] = _build(variant)
    return _CACHE[variant]


def kernel(t_inters, weights, t_near, t_far, _variant=None, _trace=False):
    from concourse.bass_utils import run_bass_kernel_spmd

    nc = _get_nc(_variant)

    t_inters = np.ascontiguousarray(np.asarray(t_inters, dtype=np.float32))
    weights = np.ascontiguousarray(np.asarray(weights, dtype=np.float32))
    t_near = np.ascontiguousarray(np.asarray(t_near, dtype=np.float32))
    t_far = np.ascontiguousarray(np.asarray(t_far, dtype=np.float32))

    in_maps = []
    for c in range(NCORES):
        s = slice(c * BS, (c + 1) * BS)
        in_maps.append(
            {
                "t_inters": t_inters[s],
                "weights": weights[s],
                "t_near": t_near[s],
                "t_far": t_far[s],
            }
        )

    res = run_bass_kernel_spmd(nc, in_maps, core_ids=list(range(NCORES)), trace=_trace)
    out = np.concatenate([res.results[c]["out"] for c in range(NCORES)])
    if _trace:
        return out, res
    return out
